# revision 17
# baseline (speedup 1.0000x reference)
"""ConvMultiheadAttention Trainium2 kernel (8 NeuronCores).

Sharding: core c = (batch b = c//2) x (head-group hg = c%2, 8 heads each).

Key optimizations over a straight bf16 port:
  - Key-padding-mask compaction: only unmasked key positions (<= CAP,
    host-gathered with their 3 conv taps) flow through the k/v convs,
    the scores and the AV matmuls — ~2x less work for those stages.
  - fp8 DoubleRow matmuls: q/k convs run in pure fp8e4m3 (2 contraction
    chunks per instruction at 0.5 cycles/row = 4x bf16 throughput); the
    v and out convs use a hi+lo fp8 residual split (3 products per chunk
    pair, ~1.33x bf16, quantization error ~1e-3).
  - Scores and AV stay bf16 (their error feeds the output directly).
  - Transposed scores sT[j, i] so the key bias folds into the Exp
    activation's per-partition bias; softmax denominator comes free from
    a ones-row in the AV matmul (row 64 of the [65, i] psum).
  - The out conv is software-pipelined in half-groups over an 8-bank
    PSUM ring: the halves reading attention pairs 0/1 for all 16 output
    groups issue first, hiding the last pair's normalize/quantize chain.

Scale bookkeeping (fp8 subnormal avoidance): x inputs are pre-scaled by
2^4 and conv weights by 2^13 on the host, so q_t/k_t/v_t carry 2^17,
score psums 2^34 (undone by the Exp activation's scale operand), the
AV ones-row is 2^12 making o_x = attn_out * 2^5, and the out-conv psum
carries 2^18 which the host divides out of the bf16 partials.

Host folds: attention scale + q-bias into the q-conv weights/bias;
k-bias dropped (constant per softmax row -> cancels); v-bias and o-bias
applied on the host after the gather (attention rows sum to 1).
"""

import os
import numpy as np
import ml_dtypes

F8 = ml_dtypes.float8_e4m3
BF16 = ml_dtypes.bfloat16

B, L, D = 4, 1024, 1024
NH, HD = 16, 64
KW = 3
NCORES = 8
HALF = D // 2  # channels per core half (8 heads)
SCALE = HD ** -0.5
MASK_BIAS = -30000.0

WS = 2.0 ** 13   # fp8 weight scale
XS = 2.0 ** 4    # fp8 x scale
OS = 2.0 ** 5    # o_x = attn_out * OS
ONES = 2.0 ** 17 / OS   # vT ones-row value
OUTS = WS * OS   # out-conv psum scale

_CACHE = {}


def _build_nc(capc):
    import concourse.bass as bass  # noqa: F401
    import concourse.tile as tile
    from concourse import bacc, mybir

    f32 = mybir.dt.float32
    bf16 = mybir.dt.bfloat16
    f8 = mybir.dt.float8e4
    Act = mybir.ActivationFunctionType
    DR = mybir.MatmulPerfMode.DoubleRow
    CAP = capc * 128

    nc = bacc.Bacc(
        "TRN2",
        target_bir_lowering=False,
        debug=False,
        enable_asserts=False,
        num_devices=NCORES,
    )

    # ---- DRAM I/O ----
    xq_d = nc.dram_tensor("xq", [8, 128, L], f8, kind="ExternalInput").ap()
    xk_d = nc.dram_tensor("xk", [8, 128, KW, CAP], f8, kind="ExternalInput").ap()
    xv_d = nc.dram_tensor("xv", [8, 128, 2, KW, CAP], f8, kind="ExternalInput").ap()
    wq_d = nc.dram_tensor("wq", [4, 128, KW, 8, 128], f8, kind="ExternalInput").ap()
    wk_d = nc.dram_tensor("wk", [4, 128, 8, KW, 128], f8, kind="ExternalInput").ap()
    wv_d = nc.dram_tensor("wv", [4, 128, 2, 8, KW, 128], f8, kind="ExternalInput").ap()
    wo_d = nc.dram_tensor("wo", [8, 128, KW, 2, 4, 128], f8, kind="ExternalInput").ap()
    qb_d = nc.dram_tensor("qb", [128, 4], f32, kind="ExternalInput").ap()
    jb_d = nc.dram_tensor("jb", [128, capc], f32, kind="ExternalInput").ap()
    out_d = nc.dram_tensor("out", [8, 128, L], bf16, kind="ExternalOutput").ap()

    from concourse.masks import make_identity

    # conv column chunks over the compacted width
    cchunks = [(s, min(512, CAP - s)) for s in range(0, CAP, 512)]

    with tile.TileContext(nc) as tc:
        with (
            tc.tile_pool(name="singles", bufs=1) as singles,
            tc.tile_pool(name="wpool", bufs=3) as wpool,
            tc.tile_pool(name="qk", bufs=2) as qkpool,
            tc.tile_pool(name="vpool", bufs=2) as vpool,
            tc.tile_pool(name="ppool", bufs=2) as ppool,
            tc.tile_pool(name="smalls", bufs=4) as smalls,
        ):
            # ---- constants / resident tiles ----
            ident = singles.tile([128, 128], bf16, tag="ident")
            make_identity(nc, ident)
            qb_s = singles.tile([128, 4], f32, tag="qb")
            jb_s = singles.tile([128, capc], f32, tag="jb")

            xq_s = singles.tile([128, 8, L], f8, tag="xq")
            xk_s = singles.tile([128, 8, KW, CAP], f8, tag="xk")
            xv_s = singles.tile([128, 2, 8, KW, CAP], f8, tag="xv")

            # vT[j_part, j_chunk, head, 0:64] = 2^17 * v[h*64+d, j];
            # col 64 = 2^12 (denominator row)
            vT = singles.tile([128, capc, 8, 65], bf16, tag="vT")
            for h in range(8):
                nc.vector.memset(vT[:, :, h, 64:65], ONES)
            # o_x*[ch, {hi,lo}, t%2, l] = fp8 split of attn_out * 2^5 for
            # attention pairs (0,1) / (2,3) — two tiles so the out conv's
            # first halves don't depend on the last pair's normalize chain
            o_xA = singles.tile([128, 2, 2, L], f8, tag="oxA")
            o_xB = singles.tile([128, 2, 2, L], f8, tag="oxB")

            # ---- DMA issue order = consumption order ----
            wq_t0 = wpool.tile([128, KW, 8, 128], f8, tag="wq", name="wq0")
            nc.sync.dma_start(wq_t0, wq_d[0])
            for cc in range(8):
                nc.sync.dma_start(xq_s[:, cc, :], xq_d[cc])
                if cc == 1:
                    nc.sync.dma_start(qb_s, qb_d)
                    nc.sync.dma_start(jb_s, jb_d)
            wk_t0 = wpool.tile([128, 8, KW, 128], f8, tag="wk", name="wk0")
            nc.sync.dma_start(wk_t0, wk_d[0])
            for cc in range(8):
                nc.sync.dma_start(xk_s[:, cc, :, :], xk_d[cc])
            wv_ts = []
            for occ in range(4):
                wv_t = wpool.tile([128, 2, 8, KW, 128], f8, tag="wv",
                                  bufs=4, name=f"wv{occ}")
                nc.sync.dma_start(wv_t, wv_d[occ])
                wv_ts.append(wv_t)
            for cc in range(8):
                nc.sync.dma_start(xv_s[:, :, cc, :, :], xv_d[cc])

            # ---- conv helpers ----
            def conv_q(ps, w_t, lh):
                """Pure-fp8 q conv into ps[:, 0:512] for output columns
                [lh*512, lh*512+512). Pairs adjacent cc chunks; cc-outer so
                compute chases the x DMAs; first matmul is the full-width
                center tap (uniform psum has_written state)."""
                first = True
                for cp in range(4):
                    for k in (1, 0, 2):
                        lo = lh * 512 + k - 1
                        lhsT = w_t[:, k, 2 * cp : 2 * cp + 2, :]
                        if lo < 0:
                            rhs = xq_s[:, 2 * cp : 2 * cp + 2, 0:511]
                            outap = ps[:, 1:512]
                        elif lo + 512 > L:
                            rhs = xq_s[:, 2 * cp : 2 * cp + 2, lo:L]
                            outap = ps[:, 0 : L - lo]
                        else:
                            rhs = xq_s[:, 2 * cp : 2 * cp + 2, lo : lo + 512]
                            outap = ps[:, 0:512]
                        nc.tensor.matmul(
                            outap, lhsT, rhs,
                            start=first, stop=(cp == 3 and k == 2),
                            perf_mode=DR,
                        )
                        first = False

            def conv_k(ps, w_t, cs, cn):
                """Pure-fp8 gathered k conv into ps[:, 0:cn] for compacted
                columns [cs, cs+cn). Taps are pre-gathered: pair (k0,k1)
                within each cc, then k2 across cc pairs."""
                first = True
                n_units = 12

                def mm(lhsT, rhs, last):
                    nonlocal first
                    nc.tensor.matmul(ps[:, 0:cn], lhsT, rhs,
                                     start=first, stop=last, perf_mode=DR)
                    first = False

                i = 0
                for cc in range(8):
                    i += 1
                    mm(w_t[:, cc, 0:2, :], xk_s[:, cc, 0:2, cs : cs + cn],
                       i == n_units)
                    if cc % 2 == 1:
                        i += 1
                        mm(w_t[:, cc - 1 : cc + 1, 2, :],
                           xk_s[:, cc - 1 : cc + 1, 2, cs : cs + cn],
                           i == n_units)

            def conv_v(ps, w_t, cs, cn):
                """Hi/lo-fp8 gathered v conv into ps[:, 0:cn]. Per (cc, k):
                one cross-term DR (xh*wl + xl*wh); hi*hi mains pair (k0,k1)
                within cc and k2 across cc pairs."""
                first = True
                n_units = 36

                def mm(lhsT, rhs, last):
                    nonlocal first
                    nc.tensor.matmul(ps[:, 0:cn], lhsT, rhs,
                                     start=first, stop=last, perf_mode=DR)
                    first = False

                i = 0
                for cc in range(8):
                    for k in range(KW):
                        i += 1
                        mm(w_t[:, 0:2, cc, k, :], xv_s[:, 0:2, cc, k, cs : cs + cn],
                           i == n_units)
                    i += 1
                    mm(w_t[:, 1, cc, 0:2, :], xv_s[:, 0, cc, 0:2, cs : cs + cn],
                       i == n_units)
                    if cc % 2 == 1:
                        i += 1
                        mm(w_t[:, 1, cc - 1 : cc + 1, 2, :],
                           xv_s[:, 0, cc - 1 : cc + 1, 2, cs : cs + cn],
                           i == n_units)

            def o_win(ps, lh, k):
                lo = lh * 512 + k - 1
                if lo < 0:
                    return slice(0, 511), ps[:, 1:512]
                if lo + 512 > L:
                    return slice(lo, L), ps[:, 0 : L - lo]
                return slice(lo, lo + 512), ps[:, 0:512]

            def conv_oA(ps, w_t, lh):
                """First half of an out-conv group: products over attention
                pairs 0/1 (o_xA). Starts the psum accumulation."""
                first = True
                for k in (1, 0, 2):
                    csl, outap = o_win(ps, lh, k)
                    for tb in range(2):
                        nc.tensor.matmul(
                            outap, w_t[:, k, 0:2, tb, :], o_xA[:, 0:2, tb, csl],
                            start=first, stop=False, perf_mode=DR)
                        first = False
                    nc.tensor.matmul(
                        outap, w_t[:, k, 1, 0:2, :], o_xA[:, 0, 0:2, csl],
                        start=False, stop=False, perf_mode=DR)

            def conv_oB(ps, w_t, lh):
                """Second half: pairs 2/3 (o_xB), pair-3 products last."""
                for k in (1, 0, 2):
                    csl, outap = o_win(ps, lh, k)
                    nc.tensor.matmul(
                        outap, w_t[:, k, 0:2, 2, :], o_xB[:, 0:2, 0, csl],
                        start=False, stop=False, perf_mode=DR)
                for k in (1, 0, 2):
                    csl, outap = o_win(ps, lh, k)
                    nc.tensor.matmul(
                        outap, w_t[:, k, 0:2, 3, :], o_xB[:, 0:2, 1, csl],
                        start=False, stop=False, perf_mode=DR)
                    nc.tensor.matmul(
                        outap, w_t[:, k, 1, 2:4, :], o_xB[:, 0, 0:2, csl],
                        start=False, stop=(k == 2), perf_mode=DR)

            wo_ts = {}

            def wo_fetch(occ):
                wo_ts[occ] = wpool.tile([128, KW, 2, 4, 128], f8, tag="wo",
                                        bufs=5, name=f"wo{occ}")
                nc.sync.dma_start(wo_ts[occ], wo_d[occ])

            with (
                tc.tile_pool(name="convp", bufs=2, space="PSUM") as convp,
                tc.tile_pool(name="scorep", bufs=2, space="PSUM") as scorep,
                tc.tile_pool(name="avp", bufs=2, space="PSUM") as avp,
            ):
                # ---- v-conv units (interleaved into the t=0 score phase) ----
                def v_unit(occ):
                    v_sb = vpool.tile([128, CAP], bf16, tag="v")
                    for cs, cn in cchunks:
                        ps = convp.tile([128, cn], f32, tag="cp")
                        conv_v(ps, wv_ts[occ], cs, cn)
                        nc.vector.tensor_copy(v_sb[:, cs : cs + cn], ps)
                    for lb in range(capc):
                        tp = convp.tile([128, 128], bf16, tag="cp")
                        nc.tensor.transpose(tp, v_sb[:, lb * 128 : (lb + 1) * 128],
                                            ident)
                        nc.vector.tensor_copy(vT[:, lb, 2 * occ, 0:64],
                                              tp[:, 0:64])
                        nc.vector.tensor_copy(vT[:, lb, 2 * occ + 1, 0:64],
                                              tp[:, 64:128])

                # ---- per-pair q/k conv units ----
                def qk_conv_units(t):
                    q_t = qkpool.tile([128, L], bf16, tag="q", name=f"q{t}")
                    k_t = qkpool.tile([128, CAP], bf16, tag="k", name=f"k{t}")
                    state = {}

                    def unit_q(lh):
                        if lh == 0 and t > 0:
                            state["wq"] = wpool.tile([128, KW, 8, 128], f8,
                                                     tag="wq", name="wqt")
                            nc.sync.dma_start(state["wq"], wq_d[t])
                        ps = convp.tile([128, 512], f32, tag="cp")
                        conv_q(ps, state["wq"] if t > 0 else wq_t0, lh)
                        nc.vector.tensor_scalar_add(
                            q_t[:, lh * 512 : (lh + 1) * 512], ps,
                            qb_s[:, t : t + 1])

                    def unit_k():
                        if t > 0:
                            state["wk"] = wpool.tile([128, 8, KW, 128], f8,
                                                     tag="wk", name="wkt")
                            nc.sync.dma_start(state["wk"], wk_d[t])
                        for cs, cn in cchunks:
                            ps = convp.tile([128, cn], f32, tag="cp")
                            conv_k(ps, state["wk"] if t > 0 else wk_t0, cs, cn)
                            nc.vector.tensor_copy(k_t[:, cs : cs + cn], ps)

                    units = [lambda: unit_q(0), lambda: unit_q(1), unit_k]
                    return q_t, k_t, units

                q_t, k_t, units0 = qk_conv_units(0)
                for u in units0:
                    u()

                # ---- attention pairs ----
                for t in range(4):
                    nq_t = nk_t = None
                    if t == 0:
                        fillers = [lambda occ=occ: v_unit(occ)
                                   for occ in range(4)]
                    elif t < 3:
                        nq_t, nk_t, fillers = qk_conv_units(t + 1)
                    else:
                        fillers = []
                        wo_fetch(0)
                        wo_fetch(1)
                    o_xt = o_xA if t < 2 else o_xB
                    tb = t % 2
                    # scores + exp per j-chunk; fillers keep PE busy while
                    # the ACT engine drains the exps
                    p_pair = [ppool.tile([128, capc, L], bf16, tag="p",
                                         name=f"p{hh}") for hh in range(2)]
                    for jc in range(capc):
                        sps_pair = [scorep.tile([128, L], f32, tag="score",
                                                name=f"sps{hh}")
                                    for hh in range(2)]
                        for ih in range(2):
                            for hh in range(2):
                                base = hh * 64
                                nc.tensor.matmul(
                                    sps_pair[hh][:, ih * 512 : (ih + 1) * 512],
                                    k_t[base : base + 64,
                                        jc * 128 : (jc + 1) * 128],
                                    q_t[base : base + 64,
                                        ih * 512 : (ih + 1) * 512],
                                    start=True, stop=True,
                                )
                        for hh in range(2):
                            nc.scalar.activation(
                                p_pair[hh][:, jc, :], sps_pair[hh], Act.Exp,
                                bias=jb_s[:, jc : jc + 1], scale=2.0 ** -34,
                            )
                        if fillers:
                            fillers.pop(0)()
                    while fillers:
                        fillers.pop(0)()
                    # AV + normalize + fp8 hi/lo split of o_x; hh=1 first —
                    # its chain has an extra DMA hop (partition base 64), so
                    # the last-finishing chain is the shorter hh=0 one
                    for hh in (1, 0):
                        h = 2 * t + hh
                        base = hh * 64
                        for ih in range(2):
                            avps = avp.tile([65, 512], f32, tag="av")
                            for jc in range(capc):
                                nc.tensor.matmul(
                                    avps, vT[:, jc, h, :],
                                    p_pair[hh][:, jc,
                                               ih * 512 : (ih + 1) * 512],
                                    start=(jc == 0), stop=(jc == capc - 1),
                                )
                            r_t = smalls.tile([1, 512], f32, tag="r")
                            nc.vector.reciprocal(r_t, avps[64:65, :])
                            bc_t = smalls.tile([64, 512], f32, tag="bc")
                            nc.gpsimd.partition_broadcast(bc_t, r_t)
                            t32 = smalls.tile([64, 512], f32, tag="t32")
                            nc.vector.tensor_mul(t32, avps[0:64, :], bc_t)
                            csl = slice(ih * 512, (ih + 1) * 512)
                            if hh == 0:
                                oh_dst = o_xt[0:64, 0, tb, csl]
                                ol_dst = o_xt[0:64, 1, tb, csl]
                                nc.scalar.activation(oh_dst, t32, Act.Copy)
                                nc.gpsimd.tensor_sub(ol_dst, t32, oh_dst)
                            else:
                                oh_t = smalls.tile([64, 512], f8, tag="oh")
                                ol_t = smalls.tile([64, 512], f8, tag="ol")
                                nc.scalar.activation(oh_t, t32, Act.Copy)
                                nc.gpsimd.tensor_sub(ol_t, t32, oh_t)
                                nc.sync.dma_start(
                                    o_xt[base : base + 64, 0, tb, csl], oh_t)
                                nc.sync.dma_start(
                                    o_xt[base : base + 64, 1, tb, csl], ol_t)
                    if t == 0:
                        # conv for pair 1 runs after AV(0) (v-conv filled
                        # the exp-drain slot this round)
                        q_t, k_t, units1 = qk_conv_units(1)
                        for u in units1:
                            u()
                    elif t < 3:
                        q_t, k_t = nq_t, nk_t

            # ---- out conv, software-pipelined in half-groups over an
            # 8-bank psum ring (partial over this core's 512 channels) ----
            with tc.tile_pool(name="opool", bufs=8, space="PSUM") as opool:
                DEPTH = 7
                groups = [(occ, lh) for occ in range(8) for lh in range(2)]
                open_ps = {}

                def finish(i):
                    occ, lh = groups[i]
                    ps = open_ps.pop(i)
                    conv_oB(ps, wo_ts[occ], lh)
                    o_t = smalls.tile([128, 512], bf16, tag="osb", bufs=2)
                    nc.vector.tensor_copy(o_t, ps)
                    nc.sync.dma_start(
                        out_d[occ, :, lh * 512 : (lh + 1) * 512], o_t)

                for i, (occ, lh) in enumerate(groups):
                    if occ not in wo_ts:
                        wo_fetch(occ)
                    ps = opool.tile([128, 512], f32, tag="op", bufs=8)
                    open_ps[i] = ps
                    conv_oA(ps, wo_ts[occ], lh)
                    if i >= DEPTH:
                        finish(i - DEPTH)
                for i in range(len(groups) - DEPTH, len(groups)):
                    finish(i)

    nc.compile()
    return nc


def _get_nc(capc=4):
    if capc not in _CACHE:
        _CACHE[capc] = _build_nc(capc)
    return _CACHE[capc]


def _f8_hl(x):
    h = np.asarray(x, np.float32).astype(F8)
    l = (np.asarray(x, np.float32) - h.astype(np.float32)).astype(F8)
    return h, l


def _prep_inputs(query, key, value, key_padding_mask, attn_mask,
                 q_w, q_b, k_w, k_b, v_w, v_b, o_w, o_b):
    """Build the 8 per-core input maps (host-side shard + layout + fp8)."""
    query = np.asarray(query, np.float32)
    key = np.asarray(key, np.float32)
    value = np.asarray(value, np.float32)
    kpm = np.asarray(key_padding_mask)
    attn_mask = np.asarray(attn_mask, np.float32)
    q_w = np.asarray(q_w, np.float32); q_b = np.asarray(q_b, np.float32)
    k_w = np.asarray(k_w, np.float32)
    v_w = np.asarray(v_w, np.float32)
    o_w = np.asarray(o_w, np.float32); o_b = np.asarray(o_b, np.float32)

    # attn_mask must be constant across query rows to fold into the key bias
    if not np.all(attn_mask == attn_mask[0:1, :]):
        raise NotImplementedError("attn_mask varying over query index unsupported")
    am_row = attn_mask[0]

    # compacted key positions per batch, shared capacity
    pos_b = [np.nonzero(~kpm[b])[0] for b in range(B)]
    n_max = max(max((len(p) for p in pos_b), default=1), 1)
    capc = (n_max + 127) // 128
    CAP = capc * 128

    def conv_w_q(w):
        # [co 512, ci 1024, K] -> [t, p(ci), k, cc, m(co)] fp8 * WS
        arr = (w * WS).reshape(4, 128, 8, 128, KW).transpose(0, 3, 4, 2, 1)
        return np.ascontiguousarray(arr).astype(F8)

    def conv_w_k(w):
        # -> [t, p(ci), cc, k, m] fp8 * WS
        arr = (w * WS).reshape(4, 128, 8, 128, KW).transpose(0, 3, 2, 4, 1)
        return np.ascontiguousarray(arr).astype(F8)

    def conv_w_v(w):
        # -> [t, p(ci), hl(lo,hi), cc, k, m] fp8 * WS
        arr = (w * WS).reshape(4, 128, 8, 128, KW).transpose(0, 3, 2, 4, 1)
        h, l = _f8_hl(arr)
        out = np.stack([l, h], axis=2)  # [t, p, 2, cc, k, m]
        return np.ascontiguousarray(out)

    def conv_w_o(w):
        # w [1024 co, 512 ci, K] -> [occ, p(ci128), k, hl(lo,hi), t, m]
        arr = (w * WS).reshape(8, 128, 4, 128, KW).transpose(0, 3, 4, 2, 1)
        h, l = _f8_hl(arr)
        out = np.stack([l, h], axis=3)  # [occ, p, k, 2, t, m]
        return np.ascontiguousarray(out)

    wq_h, wk_h, wv_h, wo_h, qb_h = [], [], [], [], []
    for hg in range(2):
        sl = slice(hg * HALF, (hg + 1) * HALF)
        wq_h.append(conv_w_q(q_w[sl] * SCALE))
        wk_h.append(conv_w_k(k_w[sl]))
        wv_h.append(conv_w_v(v_w[sl]))
        wo_h.append(conv_w_o(o_w[:, sl, :]))
        qb_h.append(np.ascontiguousarray(
            (q_b[sl] * SCALE * WS * XS).reshape(4, 128).T).astype(np.float32))

    xq_b, xk_b, xv_b, jb_b = [], [], [], []
    for b in range(B):
        pos = pos_b[b]
        n = len(pos)
        qT = query[b].T * XS  # [D, L]
        xq_b.append(np.ascontiguousarray(
            qT.reshape(8, 128, L)).astype(F8))
        # gathered taps: pad x by one column each side, index pos + k
        kT = np.pad(key[b].T * XS, ((0, 0), (1, 1)))
        vT_ = np.pad(value[b].T * XS, ((0, 0), (1, 1)))
        kg = np.zeros((D, KW, CAP), np.float32)
        vg = np.zeros((D, KW, CAP), np.float32)
        for k in range(KW):
            kg[:, k, :n] = kT[:, pos + k]
            vg[:, k, :n] = vT_[:, pos + k]
        xk_b.append(np.ascontiguousarray(
            kg.reshape(8, 128, KW, CAP)).astype(F8))
        vh, vl = _f8_hl(vg.reshape(8, 128, KW, CAP))
        xv_b.append(np.ascontiguousarray(
            np.stack([vh, vl], axis=2)))  # [8, 128, 2, KW, CAP]
        jb = np.full(CAP, MASK_BIAS, np.float32)
        jb[:n] = am_row[pos]
        jb_b.append(np.ascontiguousarray(
            jb.reshape(capc, 128).T).astype(np.float32))

    in_maps = []
    for c in range(NCORES):
        b, hg = c // 2, c % 2
        in_maps.append({
            "xq": xq_b[b], "xk": xk_b[b], "xv": xv_b[b],
            "wq": wq_h[hg], "wk": wk_h[hg], "wv": wv_h[hg], "wo": wo_h[hg],
            "qb": qb_h[hg], "jb": jb_b[b],
        })
    return in_maps, capc, (o_w, np.asarray(v_b, np.float32), o_b)


def _postprocess(parts, extras):
    """parts: list of 8 arrays [8,128,L] bf16 (x OUTS) -> [B, L, D] f32."""
    o_w, v_b, o_b = extras
    # v-bias contribution through the out conv (attention rows sum to 1):
    # interior columns see all 3 taps, edge columns lose one.
    a_full = o_w.sum(axis=2) @ v_b            # [D]
    a_l0 = a_full - o_w[:, :, 0] @ v_b        # l = 0 loses tap k=0
    a_lL = a_full - o_w[:, :, 2] @ v_b        # l = L-1 loses tap k=2
    inv = 1.0 / OUTS
    out = np.empty((B, L, D), np.float32)
    for b in range(B):
        tot = (np.asarray(parts[2 * b], np.float32)
               + np.asarray(parts[2 * b + 1], np.float32)).reshape(D, L) * inv
        tot = tot + o_b[:, None] + a_full[:, None]
        tot[:, 0] += a_l0 - a_full
        tot[:, -1] += a_lL - a_full
        out[b] = tot.T
    return out


def _run(nc, in_maps, trace=False, **kw):
    from concourse import bass_utils
    try:
        res = bass_utils.run_bass_kernel_spmd(
            nc, in_maps, core_ids=list(range(NCORES)), trace=trace, **kw)
    except ModuleNotFoundError:
        # NTFF profiling hook unavailable (axon client without axon.trn);
        # rerun without trace.
        res = bass_utils.run_bass_kernel_spmd(
            nc, in_maps, core_ids=list(range(NCORES)), trace=False, **kw)
    return res


def kernel(**inputs) -> np.ndarray:
    in_maps, capc, extras = _prep_inputs(**inputs)
    nc = _get_nc(capc)
    res = _run(nc, in_maps,
               trace=bool(int(os.environ.get("KERNEL_TRACE", "0"))))
    parts = [res.results[c]["out"] for c in range(NCORES)]
    out = _postprocess(parts, extras)
    if res.exec_time_ns is not None:
        print(f"HW exec time: {res.exec_time_ns} ns")
    return out


# revision 24
# speedup vs baseline: 1.0354x; 1.0354x over previous
"""ConvMultiheadAttention Trainium2 kernel (8 NeuronCores).

Sharding: core c = (batch b = c//2) x (head-group hg = c%2, 8 heads each).

Key optimizations over a straight bf16 port:
  - Key-padding-mask compaction: only unmasked key positions (<= CAP,
    host-gathered with their 3 conv taps) flow through the k/v convs,
    the scores and the AV matmuls — ~2x less work for those stages.
  - fp8 DoubleRow matmuls: q/k convs run in pure fp8e4m3 (2 contraction
    chunks per instruction at 0.5 cycles/row = 4x bf16 throughput); the
    v and out convs use a hi+lo fp8 residual split (3 products per chunk
    pair, ~1.33x bf16, quantization error ~1e-3).
  - Scores and AV stay bf16 (their error feeds the output directly).
  - Transposed scores sT[j, i] so the key bias folds into the Exp
    activation's per-partition bias; softmax denominator comes free from
    a ones-row in the AV matmul (row 64 of the [65, i] psum).
  - The out conv is software-pipelined in half-groups over an 8-bank
    PSUM ring: the halves reading attention pairs 0/1 for all 16 output
    groups issue first, hiding the last pair's normalize/quantize chain.

Scale bookkeeping (fp8 subnormal avoidance): x inputs are pre-scaled by
2^4 and conv weights by 2^13 on the host, so q_t/k_t/v_t carry 2^17,
score psums 2^34 (undone by the Exp activation's scale operand), the
AV ones-row is 2^12 making o_x = attn_out * 2^5, and the out-conv psum
carries 2^18 which the host divides out of the bf16 partials.

Host folds: attention scale + q-bias into the q-conv weights/bias;
k-bias dropped (constant per softmax row -> cancels); v-bias and o-bias
applied on the host after the gather (attention rows sum to 1).
"""

import os
import numpy as np
import ml_dtypes

F8 = ml_dtypes.float8_e4m3
BF16 = ml_dtypes.bfloat16

B, L, D = 4, 1024, 1024
NH, HD = 16, 64
KW = 3
NCORES = 8
HALF = D // 2  # channels per core half (8 heads)
SCALE = HD ** -0.5
MASK_BIAS = -30000.0

WS = 2.0 ** 13   # fp8 weight scale
XS = 2.0 ** 4    # fp8 x scale
OS = 2.0 ** 5    # o_x = attn_out * OS
ONES = 2.0 ** 17 / OS   # vT ones-row value
OUTS = WS * OS   # out-conv psum scale

_CACHE = {}


def _build_nc(capc):
    import concourse.bass as bass  # noqa: F401
    import concourse.tile as tile
    from concourse import bacc, mybir

    f32 = mybir.dt.float32
    bf16 = mybir.dt.bfloat16
    f8 = mybir.dt.float8e4
    Act = mybir.ActivationFunctionType
    DR = mybir.MatmulPerfMode.DoubleRow
    CAP = capc * 128

    nc = bacc.Bacc(
        "TRN2",
        target_bir_lowering=False,
        debug=False,
        enable_asserts=False,
        num_devices=NCORES,
    )

    # ---- DRAM I/O ----
    xq_d = nc.dram_tensor("xq", [8, 128, L], f8, kind="ExternalInput").ap()
    xk_d = nc.dram_tensor("xk", [8, 128, KW, CAP], f8, kind="ExternalInput").ap()
    xv_d = nc.dram_tensor("xv", [8, 128, 2, KW, CAP], f8, kind="ExternalInput").ap()
    wq_d = nc.dram_tensor("wq", [4, 128, KW, 8, 128], f8, kind="ExternalInput").ap()
    wk_d = nc.dram_tensor("wk", [4, 128, 8, KW, 128], f8, kind="ExternalInput").ap()
    wv_d = nc.dram_tensor("wv", [4, 128, 2, 8, KW, 128], f8, kind="ExternalInput").ap()
    wo_d = nc.dram_tensor("wo", [8, 128, KW, 2, 4, 128], f8, kind="ExternalInput").ap()
    qb_d = nc.dram_tensor("qb", [128, 4], f32, kind="ExternalInput").ap()
    jb_d = nc.dram_tensor("jb", [128, capc], f32, kind="ExternalInput").ap()
    out_d = nc.dram_tensor("out", [8, 128, L], bf16, kind="ExternalOutput").ap()

    from concourse.masks import make_identity

    # conv column chunks over the compacted width
    cchunks = [(s, min(512, CAP - s)) for s in range(0, CAP, 512)]

    with tile.TileContext(nc) as tc:
        with (
            tc.tile_pool(name="singles", bufs=1) as singles,
            tc.tile_pool(name="wpool", bufs=3) as wpool,
            tc.tile_pool(name="qk", bufs=2) as qkpool,
            tc.tile_pool(name="vpool", bufs=2) as vpool,
            tc.tile_pool(name="ppool", bufs=2) as ppool,
            tc.tile_pool(name="smalls", bufs=4) as smalls,
            tc.tile_pool(name="convp", bufs=2, space="PSUM") as convp,
        ):
            # ---- constants / resident tiles ----
            ident = singles.tile([128, 128], bf16, tag="ident")
            make_identity(nc, ident)
            qb_s = singles.tile([128, 4], f32, tag="qb")
            jb_s = singles.tile([128, capc], f32, tag="jb")

            xq_s = singles.tile([128, 8, L], f8, tag="xq")
            xk_s = singles.tile([128, 8, KW, CAP], f8, tag="xk")
            xv_s = singles.tile([128, 2, 8, KW, CAP], f8, tag="xv")

            # vT[j_part, j_chunk, head, 0:64] = 2^17 * v[h*64+d, j];
            # col 64 = 2^12 (denominator row)
            vT = singles.tile([128, capc, 8, 65], bf16, tag="vT")
            for h in range(8):
                nc.vector.memset(vT[:, :, h, 64:65], ONES)
            # o_x*[ch, {hi,lo}, t%2, l] = fp8 split of attn_out * 2^5 for
            # attention pairs (0,1) / (2,3) — two tiles so the out conv's
            # first halves don't depend on the last pair's normalize chain
            o_xA = singles.tile([128, 2, 2, L], f8, tag="oxA")
            o_xB = singles.tile([128, 2, 2, L], f8, tag="oxB")

            # ---- DMA issue order = consumption order ----
            wq_t0 = wpool.tile([128, KW, 8, 128], f8, tag="wq", name="wq0")
            nc.sync.dma_start(wq_t0, wq_d[0])
            for cc in range(8):
                nc.sync.dma_start(xq_s[:, cc, :], xq_d[cc])
                if cc == 1:
                    nc.sync.dma_start(qb_s, qb_d)
                    nc.sync.dma_start(jb_s, jb_d)
            wk_t0 = wpool.tile([128, 8, KW, 128], f8, tag="wk", name="wk0")
            nc.sync.dma_start(wk_t0, wk_d[0])
            for cc in range(8):
                nc.sync.dma_start(xk_s[:, cc, :, :], xk_d[cc])
            wv_ts = []
            for occ in range(4):
                wv_t = wpool.tile([128, 2, 8, KW, 128], f8, tag="wv",
                                  bufs=4, name=f"wv{occ}")
                nc.sync.dma_start(wv_t, wv_d[occ])
                wv_ts.append(wv_t)
            for cc in range(8):
                nc.sync.dma_start(xv_s[:, :, cc, :, :], xv_d[cc])

            # ---- conv helpers ----
            def conv_q(ps, w_t, lh):
                """Pure-fp8 q conv into ps[:, 0:512] for output columns
                [lh*512, lh*512+512). Pairs adjacent cc chunks; cc-outer so
                compute chases the x DMAs; first matmul is the full-width
                center tap (uniform psum has_written state)."""
                first = True
                for cp in range(4):
                    for k in (1, 0, 2):
                        lo = lh * 512 + k - 1
                        lhsT = w_t[:, k, 2 * cp : 2 * cp + 2, :]
                        if lo < 0:
                            rhs = xq_s[:, 2 * cp : 2 * cp + 2, 0:511]
                            outap = ps[:, 1:512]
                        elif lo + 512 > L:
                            rhs = xq_s[:, 2 * cp : 2 * cp + 2, lo:L]
                            outap = ps[:, 0 : L - lo]
                        else:
                            rhs = xq_s[:, 2 * cp : 2 * cp + 2, lo : lo + 512]
                            outap = ps[:, 0:512]
                        nc.tensor.matmul(
                            outap, lhsT, rhs,
                            start=first, stop=(cp == 3 and k == 2),
                            perf_mode=DR,
                        )
                        first = False

            def conv_k(ps, w_t, cs, cn):
                """Pure-fp8 gathered k conv into ps[:, 0:cn] for compacted
                columns [cs, cs+cn). Taps are pre-gathered: pair (k0,k1)
                within each cc, then k2 across cc pairs."""
                first = True
                n_units = 12

                def mm(lhsT, rhs, last):
                    nonlocal first
                    nc.tensor.matmul(ps[:, 0:cn], lhsT, rhs,
                                     start=first, stop=last, perf_mode=DR)
                    first = False

                i = 0
                for cc in range(8):
                    i += 1
                    mm(w_t[:, cc, 0:2, :], xk_s[:, cc, 0:2, cs : cs + cn],
                       i == n_units)
                    if cc % 2 == 1:
                        i += 1
                        mm(w_t[:, cc - 1 : cc + 1, 2, :],
                           xk_s[:, cc - 1 : cc + 1, 2, cs : cs + cn],
                           i == n_units)

            def conv_v(ps, w_t, cs, cn):
                """Hi/lo-fp8 gathered v conv into ps[:, 0:cn]. Per (cc, k):
                one cross-term DR (xh*wl + xl*wh); hi*hi mains pair (k0,k1)
                within cc and k2 across cc pairs."""
                first = True
                n_units = 36

                def mm(lhsT, rhs, last):
                    nonlocal first
                    nc.tensor.matmul(ps[:, 0:cn], lhsT, rhs,
                                     start=first, stop=last, perf_mode=DR)
                    first = False

                i = 0
                for cc in range(8):
                    for k in range(KW):
                        i += 1
                        mm(w_t[:, 0:2, cc, k, :], xv_s[:, 0:2, cc, k, cs : cs + cn],
                           i == n_units)
                    i += 1
                    mm(w_t[:, 1, cc, 0:2, :], xv_s[:, 0, cc, 0:2, cs : cs + cn],
                       i == n_units)
                    if cc % 2 == 1:
                        i += 1
                        mm(w_t[:, 1, cc - 1 : cc + 1, 2, :],
                           xv_s[:, 0, cc - 1 : cc + 1, 2, cs : cs + cn],
                           i == n_units)

            def o_win(ps, lh, k):
                lo = lh * 512 + k - 1
                if lo < 0:
                    return slice(0, 511), ps[:, 1:512]
                if lo + 512 > L:
                    return slice(lo, L), ps[:, 0 : L - lo]
                return slice(lo, lo + 512), ps[:, 0:512]

            def conv_oA(ps, w_t, lh):
                """First half of an out-conv group: products over attention
                pairs 0/1 (o_xA). Starts the psum accumulation."""
                first = True
                for k in (1, 0, 2):
                    csl, outap = o_win(ps, lh, k)
                    for tb in range(2):
                        nc.tensor.matmul(
                            outap, w_t[:, k, 0:2, tb, :], o_xA[:, 0:2, tb, csl],
                            start=first, stop=False, perf_mode=DR)
                        first = False
                    nc.tensor.matmul(
                        outap, w_t[:, k, 1, 0:2, :], o_xA[:, 0, 0:2, csl],
                        start=False, stop=False, perf_mode=DR)

            def conv_oB(ps, w_t, lh):
                """Second half: pairs 2/3 (o_xB), pair-3 products last."""
                for k in (1, 0, 2):
                    csl, outap = o_win(ps, lh, k)
                    nc.tensor.matmul(
                        outap, w_t[:, k, 0:2, 2, :], o_xB[:, 0:2, 0, csl],
                        start=False, stop=False, perf_mode=DR)
                for k in (1, 0, 2):
                    csl, outap = o_win(ps, lh, k)
                    nc.tensor.matmul(
                        outap, w_t[:, k, 0:2, 3, :], o_xB[:, 0:2, 1, csl],
                        start=False, stop=False, perf_mode=DR)
                    nc.tensor.matmul(
                        outap, w_t[:, k, 1, 2:4, :], o_xB[:, 0, 0:2, csl],
                        start=False, stop=(k == 2), perf_mode=DR)

            wo_ts = {}

            def wo_fetch(occ):
                wo_ts[occ] = wpool.tile([128, KW, 2, 4, 128], f8, tag="wo",
                                        bufs=5, name=f"wo{occ}")
                nc.sync.dma_start(wo_ts[occ], wo_d[occ])

            # o-conv half-group emitters; the first two A-halves double as
            # PE fillers during the last attention pair's exp drain (their
            # psums live in the long-lived convp pool)
            open_ps = {}
            groups = [(occ, lh) for occ in range(8) for lh in range(2)]

            def emit_A(i, pool, bufs, tag="op"):
                occ, lh = groups[i]
                if occ not in wo_ts:
                    wo_fetch(occ)
                ps = pool.tile([128, 512], f32, tag=tag, bufs=bufs)
                open_ps[i] = ps
                conv_oA(ps, wo_ts[occ], lh)

            def emit_B(i):
                occ, lh = groups[i]
                ps = open_ps.pop(i)
                conv_oB(ps, wo_ts[occ], lh)
                o_t = smalls.tile([128, 512], bf16, tag="osb", bufs=4)
                nc.vector.tensor_copy(o_t, ps)
                nc.sync.dma_start(
                    out_d[occ, :, lh * 512 : (lh + 1) * 512], o_t)

            with (
                tc.tile_pool(name="scorep", bufs=2, space="PSUM") as scorep,
                tc.tile_pool(name="avp", bufs=2, space="PSUM") as avp,
            ):
                # ---- v-conv units (interleaved into the t=0 score phase) ----
                def v_unit(occ):
                    v_sb = vpool.tile([128, CAP], bf16, tag="v")
                    for cs, cn in cchunks:
                        ps = convp.tile([128, cn], f32, tag="cp")
                        conv_v(ps, wv_ts[occ], cs, cn)
                        nc.vector.tensor_copy(v_sb[:, cs : cs + cn], ps)
                    for lb in range(capc):
                        tp = convp.tile([128, 128], bf16, tag="cp")
                        nc.tensor.transpose(tp, v_sb[:, lb * 128 : (lb + 1) * 128],
                                            ident)
                        nc.vector.tensor_copy(vT[:, lb, 2 * occ, 0:64],
                                              tp[:, 0:64])
                        nc.vector.tensor_copy(vT[:, lb, 2 * occ + 1, 0:64],
                                              tp[:, 64:128])

                # ---- per-pair q/k conv units ----
                def qk_conv_units(t):
                    q_t = qkpool.tile([128, L], bf16, tag="q", name=f"q{t}")
                    k_t = qkpool.tile([128, CAP], bf16, tag="k", name=f"k{t}")
                    state = {}

                    def unit_q(lh):
                        if lh == 0 and t > 0:
                            state["wq"] = wpool.tile([128, KW, 8, 128], f8,
                                                     tag="wq", name="wqt")
                            nc.sync.dma_start(state["wq"], wq_d[t])
                        ps = convp.tile([128, 512], f32, tag="cp")
                        conv_q(ps, state["wq"] if t > 0 else wq_t0, lh)
                        nc.vector.tensor_scalar_add(
                            q_t[:, lh * 512 : (lh + 1) * 512], ps,
                            qb_s[:, t : t + 1])

                    def unit_k():
                        if t > 0:
                            state["wk"] = wpool.tile([128, 8, KW, 128], f8,
                                                     tag="wk", name="wkt")
                            nc.sync.dma_start(state["wk"], wk_d[t])
                        for cs, cn in cchunks:
                            ps = convp.tile([128, cn], f32, tag="cp")
                            conv_k(ps, state["wk"] if t > 0 else wk_t0, cs, cn)
                            nc.vector.tensor_copy(k_t[:, cs : cs + cn], ps)

                    units = [lambda: unit_q(0), lambda: unit_q(1), unit_k]
                    return q_t, k_t, units

                q_t, k_t, units0 = qk_conv_units(0)
                for u in units0:
                    u()

                # ---- attention pairs ----
                for t in range(4):
                    nq_t = nk_t = None
                    if t == 0:
                        fillers = [lambda occ=occ: v_unit(occ)
                                   for occ in range(4)]
                    elif t < 3:
                        nq_t, nk_t, fillers = qk_conv_units(t + 1)
                        if t == 2:
                            wo_fetch(0)
                            wo_fetch(1)
                    else:
                        wo_fetch(2)
                        wo_fetch(3)
                        fillers = [lambda: emit_A(0, convp, 2, tag="cp"),
                                   lambda: emit_A(1, convp, 2, tag="cp")]
                    o_xt = o_xA if t < 2 else o_xB
                    tb = t % 2
                    # scores + exp per j-chunk; fillers keep PE busy while
                    # the ACT engine drains the exps
                    p_pair = [ppool.tile([128, capc, L], bf16, tag="p",
                                         name=f"p{hh}") for hh in range(2)]
                    for jc in range(capc):
                        sps_pair = [scorep.tile([128, L], f32, tag="score",
                                                name=f"sps{hh}")
                                    for hh in range(2)]
                        for ih in range(2):
                            for hh in range(2):
                                base = hh * 64
                                nc.tensor.matmul(
                                    sps_pair[hh][:, ih * 512 : (ih + 1) * 512],
                                    k_t[base : base + 64,
                                        jc * 128 : (jc + 1) * 128],
                                    q_t[base : base + 64,
                                        ih * 512 : (ih + 1) * 512],
                                    start=True, stop=True,
                                )
                        for hh in range(2):
                            nc.scalar.activation(
                                p_pair[hh][:, jc, :], sps_pair[hh], Act.Exp,
                                bias=jb_s[:, jc : jc + 1], scale=2.0 ** -34,
                            )
                        if fillers:
                            fillers.pop(0)()
                    while fillers:
                        fillers.pop(0)()
                    # AV + normalize + fp8 hi/lo split of o_x; hh=1 first —
                    # its chain has an extra DMA hop (partition base 64), so
                    # the last-finishing chain is the shorter hh=0 one
                    for hh in (1, 0):
                        h = 2 * t + hh
                        base = hh * 64
                        for ih in range(2):
                            avps = avp.tile([65, 512], f32, tag="av")
                            for jc in range(capc):
                                nc.tensor.matmul(
                                    avps, vT[:, jc, h, :],
                                    p_pair[hh][:, jc,
                                               ih * 512 : (ih + 1) * 512],
                                    start=(jc == 0), stop=(jc == capc - 1),
                                )
                            r_t = smalls.tile([1, 512], f32, tag="r")
                            nc.vector.reciprocal(r_t, avps[64:65, :])
                            bc_t = smalls.tile([64, 512], f32, tag="bc")
                            nc.gpsimd.partition_broadcast(bc_t, r_t)
                            t32 = smalls.tile([64, 512], f32, tag="t32")
                            nc.vector.tensor_mul(t32, avps[0:64, :], bc_t)
                            csl = slice(ih * 512, (ih + 1) * 512)
                            if hh == 0:
                                oh_dst = o_xt[0:64, 0, tb, csl]
                                ol_dst = o_xt[0:64, 1, tb, csl]
                                nc.scalar.activation(oh_dst, t32, Act.Copy)
                                nc.gpsimd.tensor_sub(ol_dst, t32, oh_dst)
                            else:
                                oh_t = smalls.tile([64, 512], f8, tag="oh")
                                ol_t = smalls.tile([64, 512], f8, tag="ol")
                                nc.scalar.activation(oh_t, t32, Act.Copy)
                                nc.gpsimd.tensor_sub(ol_t, t32, oh_t)
                                nc.sync.dma_start(
                                    o_xt[base : base + 64, 0, tb, csl], oh_t)
                                nc.sync.dma_start(
                                    o_xt[base : base + 64, 1, tb, csl], ol_t)
                    if t == 0:
                        # conv for pair 1 runs after AV(0) (v-conv filled
                        # the exp-drain slot this round)
                        q_t, k_t, units1 = qk_conv_units(1)
                        for u in units1:
                            u()
                    elif t < 3:
                        q_t, k_t = nq_t, nk_t

            # ---- out conv, software-pipelined in half-groups (partial
            # over this core's 512 attention channels). A(0)/A(1) were
            # already emitted as t=3 fillers on convp; the rest ride a
            # 6-bank opool ring, B-halves trailing by 6 so the last pair's
            # normalize chain is hidden and the stores stay spread out. ----
            with tc.tile_pool(name="opool", bufs=6, space="PSUM") as opool:
                for i in range(2, 8):
                    emit_A(i, opool, 6)
                emit_B(0)
                emit_B(1)
                for i in range(8, 16):
                    emit_B(i - 6)
                    emit_A(i, opool, 6)
                for i in range(10, 16):
                    emit_B(i)

    nc.compile()
    return nc


def _get_nc(capc=4):
    if capc not in _CACHE:
        _CACHE[capc] = _build_nc(capc)
    return _CACHE[capc]


def _f8_hl(x):
    h = np.asarray(x, np.float32).astype(F8)
    l = (np.asarray(x, np.float32) - h.astype(np.float32)).astype(F8)
    return h, l


def _prep_inputs(query, key, value, key_padding_mask, attn_mask,
                 q_w, q_b, k_w, k_b, v_w, v_b, o_w, o_b):
    """Build the 8 per-core input maps (host-side shard + layout + fp8)."""
    query = np.asarray(query, np.float32)
    key = np.asarray(key, np.float32)
    value = np.asarray(value, np.float32)
    kpm = np.asarray(key_padding_mask)
    attn_mask = np.asarray(attn_mask, np.float32)
    q_w = np.asarray(q_w, np.float32); q_b = np.asarray(q_b, np.float32)
    k_w = np.asarray(k_w, np.float32)
    v_w = np.asarray(v_w, np.float32)
    o_w = np.asarray(o_w, np.float32); o_b = np.asarray(o_b, np.float32)

    # attn_mask must be constant across query rows to fold into the key bias
    if not np.all(attn_mask == attn_mask[0:1, :]):
        raise NotImplementedError("attn_mask varying over query index unsupported")
    am_row = attn_mask[0]

    # compacted key positions per batch, shared capacity
    pos_b = [np.nonzero(~kpm[b])[0] for b in range(B)]
    n_max = max(max((len(p) for p in pos_b), default=1), 1)
    capc = (n_max + 127) // 128
    CAP = capc * 128

    def conv_w_q(w):
        # [co 512, ci 1024, K] -> [t, p(ci), k, cc, m(co)] fp8 * WS
        arr = (w * WS).reshape(4, 128, 8, 128, KW).transpose(0, 3, 4, 2, 1)
        return np.ascontiguousarray(arr).astype(F8)

    def conv_w_k(w):
        # -> [t, p(ci), cc, k, m] fp8 * WS
        arr = (w * WS).reshape(4, 128, 8, 128, KW).transpose(0, 3, 2, 4, 1)
        return np.ascontiguousarray(arr).astype(F8)

    def conv_w_v(w):
        # -> [t, p(ci), hl(lo,hi), cc, k, m] fp8 * WS
        arr = (w * WS).reshape(4, 128, 8, 128, KW).transpose(0, 3, 2, 4, 1)
        h, l = _f8_hl(arr)
        out = np.stack([l, h], axis=2)  # [t, p, 2, cc, k, m]
        return np.ascontiguousarray(out)

    def conv_w_o(w):
        # w [1024 co, 512 ci, K] -> [occ, p(ci128), k, hl(lo,hi), t, m]
        arr = (w * WS).reshape(8, 128, 4, 128, KW).transpose(0, 3, 4, 2, 1)
        h, l = _f8_hl(arr)
        out = np.stack([l, h], axis=3)  # [occ, p, k, 2, t, m]
        return np.ascontiguousarray(out)

    wq_h, wk_h, wv_h, wo_h, qb_h = [], [], [], [], []
    for hg in range(2):
        sl = slice(hg * HALF, (hg + 1) * HALF)
        wq_h.append(conv_w_q(q_w[sl] * SCALE))
        wk_h.append(conv_w_k(k_w[sl]))
        wv_h.append(conv_w_v(v_w[sl]))
        wo_h.append(conv_w_o(o_w[:, sl, :]))
        qb_h.append(np.ascontiguousarray(
            (q_b[sl] * SCALE * WS * XS).reshape(4, 128).T).astype(np.float32))

    xq_b, xk_b, xv_b, jb_b = [], [], [], []
    for b in range(B):
        pos = pos_b[b]
        n = len(pos)
        qT = query[b].T * XS  # [D, L]
        xq_b.append(np.ascontiguousarray(
            qT.reshape(8, 128, L)).astype(F8))
        # gathered taps: pad x by one column each side, index pos + k
        kT = np.pad(key[b].T * XS, ((0, 0), (1, 1)))
        vT_ = np.pad(value[b].T * XS, ((0, 0), (1, 1)))
        kg = np.zeros((D, KW, CAP), np.float32)
        vg = np.zeros((D, KW, CAP), np.float32)
        for k in range(KW):
            kg[:, k, :n] = kT[:, pos + k]
            vg[:, k, :n] = vT_[:, pos + k]
        xk_b.append(np.ascontiguousarray(
            kg.reshape(8, 128, KW, CAP)).astype(F8))
        vh, vl = _f8_hl(vg.reshape(8, 128, KW, CAP))
        xv_b.append(np.ascontiguousarray(
            np.stack([vh, vl], axis=2)))  # [8, 128, 2, KW, CAP]
        jb = np.full(CAP, MASK_BIAS, np.float32)
        jb[:n] = am_row[pos]
        jb_b.append(np.ascontiguousarray(
            jb.reshape(capc, 128).T).astype(np.float32))

    in_maps = []
    for c in range(NCORES):
        b, hg = c // 2, c % 2
        in_maps.append({
            "xq": xq_b[b], "xk": xk_b[b], "xv": xv_b[b],
            "wq": wq_h[hg], "wk": wk_h[hg], "wv": wv_h[hg], "wo": wo_h[hg],
            "qb": qb_h[hg], "jb": jb_b[b],
        })
    return in_maps, capc, (o_w, np.asarray(v_b, np.float32), o_b)


def _postprocess(parts, extras):
    """parts: list of 8 arrays [8,128,L] bf16 (x OUTS) -> [B, L, D] f32."""
    o_w, v_b, o_b = extras
    # v-bias contribution through the out conv (attention rows sum to 1):
    # interior columns see all 3 taps, edge columns lose one.
    a_full = o_w.sum(axis=2) @ v_b            # [D]
    a_l0 = a_full - o_w[:, :, 0] @ v_b        # l = 0 loses tap k=0
    a_lL = a_full - o_w[:, :, 2] @ v_b        # l = L-1 loses tap k=2
    inv = 1.0 / OUTS
    out = np.empty((B, L, D), np.float32)
    for b in range(B):
        tot = (np.asarray(parts[2 * b], np.float32)
               + np.asarray(parts[2 * b + 1], np.float32)).reshape(D, L) * inv
        tot = tot + o_b[:, None] + a_full[:, None]
        tot[:, 0] += a_l0 - a_full
        tot[:, -1] += a_lL - a_full
        out[b] = tot.T
    return out


def _run(nc, in_maps, trace=False, **kw):
    from concourse import bass_utils
    try:
        res = bass_utils.run_bass_kernel_spmd(
            nc, in_maps, core_ids=list(range(NCORES)), trace=trace, **kw)
    except ModuleNotFoundError:
        # NTFF profiling hook unavailable (axon client without axon.trn);
        # rerun without trace.
        res = bass_utils.run_bass_kernel_spmd(
            nc, in_maps, core_ids=list(range(NCORES)), trace=False, **kw)
    return res


def kernel(**inputs) -> np.ndarray:
    in_maps, capc, extras = _prep_inputs(**inputs)
    nc = _get_nc(capc)
    res = _run(nc, in_maps,
               trace=bool(int(os.environ.get("KERNEL_TRACE", "0"))))
    parts = [res.results[c]["out"] for c in range(NCORES)]
    out = _postprocess(parts, extras)
    if res.exec_time_ns is not None:
        print(f"HW exec time: {res.exec_time_ns} ns")
    return out


# revision 25
# speedup vs baseline: 1.0760x; 1.0393x over previous
"""ConvMultiheadAttention Trainium2 kernel (8 NeuronCores).

Sharding: core c = (batch b = c//2) x (head-group hg = c%2, 8 heads each).

Key optimizations over a straight bf16 port:
  - Key-padding-mask compaction: only unmasked key positions (<= CAP,
    host-gathered with their 3 conv taps) flow through the k/v convs,
    the scores and the AV matmuls — ~2x less work for those stages.
  - fp8 DoubleRow matmuls: q/k convs run in pure fp8e4m3 (2 contraction
    chunks per instruction at 0.5 cycles/row = 4x bf16 throughput); the
    v and out convs use a hi+lo fp8 residual split (3 products per chunk
    pair, ~1.33x bf16, quantization error ~1e-3).
  - Scores and AV stay bf16 (their error feeds the output directly).
  - Transposed scores sT[j, i] so the key bias folds into the Exp
    activation's per-partition bias; softmax denominator comes free from
    a ones-row in the AV matmul (row 64 of the [65, i] psum).
  - The out conv is software-pipelined in half-groups over an 8-bank
    PSUM ring: the halves reading attention pairs 0/1 for all 16 output
    groups issue first, hiding the last pair's normalize/quantize chain.

Scale bookkeeping (fp8 subnormal avoidance): x inputs are pre-scaled by
2^4 and conv weights by 2^13 on the host, so q_t/k_t/v_t carry 2^17,
score psums 2^34 (undone by the Exp activation's scale operand), the
AV ones-row is 2^12 making o_x = attn_out * 2^5, and the out-conv psum
carries 2^18 which the host divides out of the bf16 partials.

Host folds: attention scale + q-bias into the q-conv weights/bias;
k-bias dropped (constant per softmax row -> cancels); v-bias and o-bias
applied on the host after the gather (attention rows sum to 1).
"""

import os
import numpy as np
import ml_dtypes

F8 = ml_dtypes.float8_e4m3
BF16 = ml_dtypes.bfloat16

B, L, D = 4, 1024, 1024
NH, HD = 16, 64
KW = 3
NCORES = 8
HALF = D // 2  # channels per core half (8 heads)
SCALE = HD ** -0.5
MASK_BIAS = -30000.0

WS = 2.0 ** 13   # fp8 weight scale
XS = 2.0 ** 4    # fp8 x scale
OS = 2.0 ** 5    # o_x = attn_out * OS
ONES = 2.0 ** 17 / OS   # vT ones-row value
OUTS = WS * OS   # out-conv psum scale

_CACHE = {}


def _build_nc(capc):
    import concourse.bass as bass  # noqa: F401
    import concourse.tile as tile
    from concourse import bacc, mybir

    f32 = mybir.dt.float32
    bf16 = mybir.dt.bfloat16
    f8 = mybir.dt.float8e4
    Act = mybir.ActivationFunctionType
    DR = mybir.MatmulPerfMode.DoubleRow
    CAP = capc * 128

    nc = bacc.Bacc(
        "TRN2",
        target_bir_lowering=False,
        debug=False,
        enable_asserts=False,
        num_devices=NCORES,
    )

    # ---- DRAM I/O ----
    xq_d = nc.dram_tensor("xq", [8, 128, L], f8, kind="ExternalInput").ap()
    xk_d = nc.dram_tensor("xk", [8, 128, KW, CAP], f8, kind="ExternalInput").ap()
    xv_d = nc.dram_tensor("xv", [8, 128, 2, KW, CAP], f8, kind="ExternalInput").ap()
    wq_d = nc.dram_tensor("wq", [4, 128, KW, 8, 128], f8, kind="ExternalInput").ap()
    wk_d = nc.dram_tensor("wk", [4, 128, 8, KW, 128], f8, kind="ExternalInput").ap()
    wv_d = nc.dram_tensor("wv", [4, 128, 2, 8, KW, 128], f8, kind="ExternalInput").ap()
    wo_d = nc.dram_tensor("wo", [8, 128, KW, 2, 4, 128], f8, kind="ExternalInput").ap()
    qb_d = nc.dram_tensor("qb", [128, 4], f32, kind="ExternalInput").ap()
    jb_d = nc.dram_tensor("jb", [128, capc], f32, kind="ExternalInput").ap()
    out_d = nc.dram_tensor("out", [8, 128, L], bf16, kind="ExternalOutput").ap()

    from concourse.masks import make_identity

    # conv column chunks over the compacted width
    cchunks = [(s, min(512, CAP - s)) for s in range(0, CAP, 512)]

    with tile.TileContext(nc) as tc:
        with (
            tc.tile_pool(name="singles", bufs=1) as singles,
            tc.tile_pool(name="wpool", bufs=3) as wpool,
            tc.tile_pool(name="qk", bufs=2) as qkpool,
            tc.tile_pool(name="vpool", bufs=2) as vpool,
            tc.tile_pool(name="ppool", bufs=2) as ppool,
            tc.tile_pool(name="smalls", bufs=4) as smalls,
            tc.tile_pool(name="convp", bufs=2, space="PSUM") as convp,
        ):
            # ---- constants / resident tiles ----
            ident = singles.tile([128, 128], bf16, tag="ident")
            make_identity(nc, ident)
            qb_s = singles.tile([128, 4], f32, tag="qb")
            jb_s = singles.tile([128, capc], f32, tag="jb")

            xq_s = singles.tile([128, 8, L], f8, tag="xq")
            xk_s = singles.tile([128, 8, KW, CAP], f8, tag="xk")
            xv_s = singles.tile([128, 2, 8, KW, CAP], f8, tag="xv")

            # vT[j_part, j_chunk, head, 0:64] = 2^17 * v[h*64+d, j];
            # col 64 = 2^12 (denominator row)
            vT = singles.tile([128, capc, 8, 65], bf16, tag="vT")
            for h in range(8):
                nc.vector.memset(vT[:, :, h, 64:65], ONES)
            # o_x*[ch, {hi,lo}, t%2, l] = fp8 split of attn_out * 2^5 for
            # attention pairs (0,1) / (2,3) — two tiles so the out conv's
            # first halves don't depend on the last pair's normalize chain
            o_xA = singles.tile([128, 2, 2, L], f8, tag="oxA")
            o_xB = singles.tile([128, 2, 2, L], f8, tag="oxB")

            # ---- DMA issue order = consumption order ----
            wq_t0 = wpool.tile([128, KW, 8, 128], f8, tag="wq", name="wq0")
            nc.sync.dma_start(wq_t0, wq_d[0])
            for cc in range(8):
                nc.sync.dma_start(xq_s[:, cc, :], xq_d[cc])
                if cc == 1:
                    nc.sync.dma_start(qb_s, qb_d)
                    nc.sync.dma_start(jb_s, jb_d)
            wk_t0 = wpool.tile([128, 8, KW, 128], f8, tag="wk", name="wk0")
            nc.sync.dma_start(wk_t0, wk_d[0])
            for cc in range(8):
                nc.sync.dma_start(xk_s[:, cc, :, :], xk_d[cc])
            wv_ts = []
            for occ in range(4):
                wv_t = wpool.tile([128, 2, 8, KW, 128], f8, tag="wv",
                                  bufs=4, name=f"wv{occ}")
                nc.sync.dma_start(wv_t, wv_d[occ])
                wv_ts.append(wv_t)
            for cc in range(8):
                nc.sync.dma_start(xv_s[:, :, cc, :, :], xv_d[cc])

            # ---- conv helpers ----
            def conv_q(ps, w_t, lh):
                """Pure-fp8 q conv into ps[:, 0:512] for output columns
                [lh*512, lh*512+512). Pairs adjacent cc chunks; cc-outer so
                compute chases the x DMAs; first matmul is the full-width
                center tap (uniform psum has_written state)."""
                first = True
                for cp in range(4):
                    for k in (1, 0, 2):
                        lo = lh * 512 + k - 1
                        lhsT = w_t[:, k, 2 * cp : 2 * cp + 2, :]
                        if lo < 0:
                            rhs = xq_s[:, 2 * cp : 2 * cp + 2, 0:511]
                            outap = ps[:, 1:512]
                        elif lo + 512 > L:
                            rhs = xq_s[:, 2 * cp : 2 * cp + 2, lo:L]
                            outap = ps[:, 0 : L - lo]
                        else:
                            rhs = xq_s[:, 2 * cp : 2 * cp + 2, lo : lo + 512]
                            outap = ps[:, 0:512]
                        nc.tensor.matmul(
                            outap, lhsT, rhs,
                            start=first, stop=(cp == 3 and k == 2),
                            perf_mode=DR,
                        )
                        first = False

            def conv_k(ps, w_t, cs, cn):
                """Pure-fp8 gathered k conv into ps[:, 0:cn] for compacted
                columns [cs, cs+cn). Taps are pre-gathered: pair (k0,k1)
                within each cc, then k2 across cc pairs."""
                first = True
                n_units = 12

                def mm(lhsT, rhs, last):
                    nonlocal first
                    nc.tensor.matmul(ps[:, 0:cn], lhsT, rhs,
                                     start=first, stop=last, perf_mode=DR)
                    first = False

                i = 0
                for cc in range(8):
                    i += 1
                    mm(w_t[:, cc, 0:2, :], xk_s[:, cc, 0:2, cs : cs + cn],
                       i == n_units)
                    if cc % 2 == 1:
                        i += 1
                        mm(w_t[:, cc - 1 : cc + 1, 2, :],
                           xk_s[:, cc - 1 : cc + 1, 2, cs : cs + cn],
                           i == n_units)

            def conv_v(ps, w_t, cs, cn):
                """Hi/lo-fp8 gathered v conv into ps[:, 0:cn]. Per (cc, k):
                one cross-term DR (xh*wl + xl*wh); hi*hi mains pair (k0,k1)
                within cc and k2 across cc pairs."""
                first = True
                n_units = 36

                def mm(lhsT, rhs, last):
                    nonlocal first
                    nc.tensor.matmul(ps[:, 0:cn], lhsT, rhs,
                                     start=first, stop=last, perf_mode=DR)
                    first = False

                i = 0
                for cc in range(8):
                    for k in range(KW):
                        i += 1
                        mm(w_t[:, 0:2, cc, k, :], xv_s[:, 0:2, cc, k, cs : cs + cn],
                           i == n_units)
                    i += 1
                    mm(w_t[:, 1, cc, 0:2, :], xv_s[:, 0, cc, 0:2, cs : cs + cn],
                       i == n_units)
                    if cc % 2 == 1:
                        i += 1
                        mm(w_t[:, 1, cc - 1 : cc + 1, 2, :],
                           xv_s[:, 0, cc - 1 : cc + 1, 2, cs : cs + cn],
                           i == n_units)

            def o_win(ps, lh, k):
                lo = lh * 512 + k - 1
                if lo < 0:
                    return slice(0, 511), ps[:, 1:512]
                if lo + 512 > L:
                    return slice(lo, L), ps[:, 0 : L - lo]
                return slice(lo, lo + 512), ps[:, 0:512]

            def conv_oA(ps, w_t, lh):
                """First half of an out-conv group: products over attention
                pairs 0/1 (o_xA). Starts the psum accumulation."""
                first = True
                for k in (1, 0, 2):
                    csl, outap = o_win(ps, lh, k)
                    for tb in range(2):
                        nc.tensor.matmul(
                            outap, w_t[:, k, 0:2, tb, :], o_xA[:, 0:2, tb, csl],
                            start=first, stop=False, perf_mode=DR)
                        first = False
                    nc.tensor.matmul(
                        outap, w_t[:, k, 1, 0:2, :], o_xA[:, 0, 0:2, csl],
                        start=False, stop=False, perf_mode=DR)

            def conv_oB(ps, w_t, lh):
                """Second half: pairs 2/3 (o_xB), pair-3 products last."""
                for k in (1, 0, 2):
                    csl, outap = o_win(ps, lh, k)
                    nc.tensor.matmul(
                        outap, w_t[:, k, 0:2, 2, :], o_xB[:, 0:2, 0, csl],
                        start=False, stop=False, perf_mode=DR)
                for k in (1, 0, 2):
                    csl, outap = o_win(ps, lh, k)
                    nc.tensor.matmul(
                        outap, w_t[:, k, 0:2, 3, :], o_xB[:, 0:2, 1, csl],
                        start=False, stop=False, perf_mode=DR)
                    nc.tensor.matmul(
                        outap, w_t[:, k, 1, 2:4, :], o_xB[:, 0, 0:2, csl],
                        start=False, stop=(k == 2), perf_mode=DR)

            wo_ts = {}

            def wo_fetch(occ):
                wo_ts[occ] = wpool.tile([128, KW, 2, 4, 128], f8, tag="wo",
                                        bufs=5, name=f"wo{occ}")
                nc.sync.dma_start(wo_ts[occ], wo_d[occ])

            # o-conv half-group emitters; the first two A-halves double as
            # PE fillers during the last attention pair's exp drain (their
            # psums live in the long-lived convp pool)
            open_ps = {}
            groups = [(occ, lh) for occ in range(8) for lh in range(2)]

            def emit_A(i, pool, bufs, tag="op"):
                occ, lh = groups[i]
                if occ not in wo_ts:
                    wo_fetch(occ)
                ps = pool.tile([128, 512], f32, tag=tag, bufs=bufs)
                open_ps[i] = ps
                conv_oA(ps, wo_ts[occ], lh)

            def emit_B(i):
                occ, lh = groups[i]
                ps = open_ps.pop(i)
                conv_oB(ps, wo_ts[occ], lh)
                o_t = smalls.tile([128, 512], bf16, tag="osb", bufs=4)
                nc.vector.tensor_copy(o_t, ps)
                nc.sync.dma_start(
                    out_d[occ, :, lh * 512 : (lh + 1) * 512], o_t)

            with (
                tc.tile_pool(name="scorep", bufs=2, space="PSUM") as scorep,
                tc.tile_pool(name="avp", bufs=2, space="PSUM") as avp,
            ):
                # ---- v-conv units (interleaved into the t=0 score phase) ----
                def v_unit(occ):
                    v_sb = vpool.tile([128, CAP], bf16, tag="v")
                    for cs, cn in cchunks:
                        ps = convp.tile([128, cn], f32, tag="cp")
                        conv_v(ps, wv_ts[occ], cs, cn)
                        nc.vector.tensor_copy(v_sb[:, cs : cs + cn], ps)
                    for lb in range(capc):
                        tp = convp.tile([128, 128], bf16, tag="cp")
                        nc.tensor.transpose(tp, v_sb[:, lb * 128 : (lb + 1) * 128],
                                            ident)
                        nc.vector.tensor_copy(vT[:, lb, 2 * occ, 0:64],
                                              tp[:, 0:64])
                        nc.vector.tensor_copy(vT[:, lb, 2 * occ + 1, 0:64],
                                              tp[:, 64:128])

                # ---- per-pair q/k conv units ----
                def qk_conv_units(t):
                    q_t = qkpool.tile([128, L], bf16, tag="q", name=f"q{t}")
                    k_t = qkpool.tile([128, CAP], bf16, tag="k", name=f"k{t}")
                    state = {}

                    def unit_q(lh):
                        if lh == 0 and t > 0:
                            state["wq"] = wpool.tile([128, KW, 8, 128], f8,
                                                     tag="wq", name="wqt")
                            nc.sync.dma_start(state["wq"], wq_d[t])
                        ps = convp.tile([128, 512], f32, tag="cp")
                        conv_q(ps, state["wq"] if t > 0 else wq_t0, lh)
                        nc.vector.tensor_scalar_add(
                            q_t[:, lh * 512 : (lh + 1) * 512], ps,
                            qb_s[:, t : t + 1])

                    def unit_k():
                        if t > 0:
                            state["wk"] = wpool.tile([128, 8, KW, 128], f8,
                                                     tag="wk", name="wkt")
                            nc.sync.dma_start(state["wk"], wk_d[t])
                        for cs, cn in cchunks:
                            ps = convp.tile([128, cn], f32, tag="cp")
                            conv_k(ps, state["wk"] if t > 0 else wk_t0, cs, cn)
                            nc.vector.tensor_copy(k_t[:, cs : cs + cn], ps)

                    units = [lambda: unit_q(0), lambda: unit_q(1), unit_k]
                    return q_t, k_t, units

                q_t, k_t, units0 = qk_conv_units(0)
                for u in units0:
                    u()

                # ---- attention pairs ----
                for t in range(4):
                    nq_t = nk_t = None
                    if t == 0:
                        fillers = [lambda occ=occ: v_unit(occ)
                                   for occ in range(4)]
                    elif t < 3:
                        nq_t, nk_t, fillers = qk_conv_units(t + 1)
                        if t == 2:
                            wo_fetch(0)
                            wo_fetch(1)
                    else:
                        wo_fetch(2)
                        wo_fetch(3)
                        fillers = [lambda: emit_A(0, convp, 2, tag="cp"),
                                   lambda: emit_A(1, convp, 2, tag="cp")]
                    o_xt = o_xA if t < 2 else o_xB
                    tb = t % 2
                    # scores + exp per j-chunk; fillers keep PE busy while
                    # the ACT engine drains the exps
                    p_pair = [ppool.tile([128, capc, L], bf16, tag="p",
                                         name=f"p{hh}") for hh in range(2)]
                    for jc in range(capc):
                        sps_pair = [scorep.tile([128, L], f32, tag="score",
                                                name=f"sps{hh}")
                                    for hh in range(2)]
                        for ih in range(2):
                            for hh in range(2):
                                base = hh * 64
                                nc.tensor.matmul(
                                    sps_pair[hh][:, ih * 512 : (ih + 1) * 512],
                                    k_t[base : base + 64,
                                        jc * 128 : (jc + 1) * 128],
                                    q_t[base : base + 64,
                                        ih * 512 : (ih + 1) * 512],
                                    start=True, stop=True,
                                )
                        for hh in range(2):
                            nc.scalar.activation(
                                p_pair[hh][:, jc, :], sps_pair[hh], Act.Exp,
                                bias=jb_s[:, jc : jc + 1], scale=2.0 ** -34,
                            )
                        if fillers:
                            fillers.pop(0)()
                    while fillers:
                        fillers.pop(0)()
                    # AV + normalize + fp8 hi/lo split of o_x; hh=1 first —
                    # its chain has an extra DMA hop (partition base 64), so
                    # the last-finishing chain is the shorter hh=0 one
                    for hh in (1, 0):
                        h = 2 * t + hh
                        base = hh * 64
                        for ih in range(2):
                            avps = avp.tile([65, 512], f32, tag="av")
                            for jc in range(capc):
                                nc.tensor.matmul(
                                    avps, vT[:, jc, h, :],
                                    p_pair[hh][:, jc,
                                               ih * 512 : (ih + 1) * 512],
                                    start=(jc == 0), stop=(jc == capc - 1),
                                )
                            # copy the psum to SBUF right away (and recip in
                            # parallel) so the avp bank frees after ~0.7us —
                            # the rest of the chain runs from SBUF
                            av_sb = smalls.tile([64, 512], f32, tag="avsb")
                            nc.vector.tensor_copy(av_sb, avps[0:64, :])
                            r_t = smalls.tile([1, 512], f32, tag="r")
                            nc.vector.reciprocal(r_t, avps[64:65, :])
                            bc_t = smalls.tile([64, 512], f32, tag="bc")
                            nc.gpsimd.partition_broadcast(bc_t, r_t)
                            t32 = smalls.tile([64, 512], f32, tag="t32")
                            nc.vector.tensor_mul(t32, av_sb, bc_t)
                            csl = slice(ih * 512, (ih + 1) * 512)
                            if hh == 0:
                                oh_dst = o_xt[0:64, 0, tb, csl]
                                ol_dst = o_xt[0:64, 1, tb, csl]
                                nc.gpsimd.tensor_copy(oh_dst, t32)
                                nc.gpsimd.tensor_sub(ol_dst, t32, oh_dst)
                            else:
                                oh_t = smalls.tile([64, 512], f8, tag="oh")
                                ol_t = smalls.tile([64, 512], f8, tag="ol")
                                nc.gpsimd.tensor_copy(oh_t, t32)
                                nc.gpsimd.tensor_sub(ol_t, t32, oh_t)
                                nc.sync.dma_start(
                                    o_xt[base : base + 64, 0, tb, csl], oh_t)
                                nc.sync.dma_start(
                                    o_xt[base : base + 64, 1, tb, csl], ol_t)
                    if t == 0:
                        # conv for pair 1 runs after AV(0) (v-conv filled
                        # the exp-drain slot this round)
                        q_t, k_t, units1 = qk_conv_units(1)
                        for u in units1:
                            u()
                    elif t < 3:
                        q_t, k_t = nq_t, nk_t

            # ---- out conv, software-pipelined in half-groups (partial
            # over this core's 512 attention channels). A(0)/A(1) were
            # already emitted as t=3 fillers on convp; the rest ride a
            # 6-bank opool ring, B-halves trailing by 6 so the last pair's
            # normalize chain is hidden and the stores stay spread out. ----
            with tc.tile_pool(name="opool", bufs=6, space="PSUM") as opool:
                for i in range(2, 8):
                    emit_A(i, opool, 6)
                emit_B(0)
                emit_B(1)
                for i in range(8, 16):
                    emit_B(i - 6)
                    emit_A(i, opool, 6)
                for i in range(10, 16):
                    emit_B(i)

    nc.compile()
    return nc


def _get_nc(capc=4):
    if capc not in _CACHE:
        _CACHE[capc] = _build_nc(capc)
    return _CACHE[capc]


def _f8_hl(x):
    h = np.asarray(x, np.float32).astype(F8)
    l = (np.asarray(x, np.float32) - h.astype(np.float32)).astype(F8)
    return h, l


def _prep_inputs(query, key, value, key_padding_mask, attn_mask,
                 q_w, q_b, k_w, k_b, v_w, v_b, o_w, o_b):
    """Build the 8 per-core input maps (host-side shard + layout + fp8)."""
    query = np.asarray(query, np.float32)
    key = np.asarray(key, np.float32)
    value = np.asarray(value, np.float32)
    kpm = np.asarray(key_padding_mask)
    attn_mask = np.asarray(attn_mask, np.float32)
    q_w = np.asarray(q_w, np.float32); q_b = np.asarray(q_b, np.float32)
    k_w = np.asarray(k_w, np.float32)
    v_w = np.asarray(v_w, np.float32)
    o_w = np.asarray(o_w, np.float32); o_b = np.asarray(o_b, np.float32)

    # attn_mask must be constant across query rows to fold into the key bias
    if not np.all(attn_mask == attn_mask[0:1, :]):
        raise NotImplementedError("attn_mask varying over query index unsupported")
    am_row = attn_mask[0]

    # compacted key positions per batch, shared capacity
    pos_b = [np.nonzero(~kpm[b])[0] for b in range(B)]
    n_max = max(max((len(p) for p in pos_b), default=1), 1)
    capc = (n_max + 127) // 128
    CAP = capc * 128

    def conv_w_q(w):
        # [co 512, ci 1024, K] -> [t, p(ci), k, cc, m(co)] fp8 * WS
        arr = (w * WS).reshape(4, 128, 8, 128, KW).transpose(0, 3, 4, 2, 1)
        return np.ascontiguousarray(arr).astype(F8)

    def conv_w_k(w):
        # -> [t, p(ci), cc, k, m] fp8 * WS
        arr = (w * WS).reshape(4, 128, 8, 128, KW).transpose(0, 3, 2, 4, 1)
        return np.ascontiguousarray(arr).astype(F8)

    def conv_w_v(w):
        # -> [t, p(ci), hl(lo,hi), cc, k, m] fp8 * WS
        arr = (w * WS).reshape(4, 128, 8, 128, KW).transpose(0, 3, 2, 4, 1)
        h, l = _f8_hl(arr)
        out = np.stack([l, h], axis=2)  # [t, p, 2, cc, k, m]
        return np.ascontiguousarray(out)

    def conv_w_o(w):
        # w [1024 co, 512 ci, K] -> [occ, p(ci128), k, hl(lo,hi), t, m]
        arr = (w * WS).reshape(8, 128, 4, 128, KW).transpose(0, 3, 4, 2, 1)
        h, l = _f8_hl(arr)
        out = np.stack([l, h], axis=3)  # [occ, p, k, 2, t, m]
        return np.ascontiguousarray(out)

    wq_h, wk_h, wv_h, wo_h, qb_h = [], [], [], [], []
    for hg in range(2):
        sl = slice(hg * HALF, (hg + 1) * HALF)
        wq_h.append(conv_w_q(q_w[sl] * SCALE))
        wk_h.append(conv_w_k(k_w[sl]))
        wv_h.append(conv_w_v(v_w[sl]))
        wo_h.append(conv_w_o(o_w[:, sl, :]))
        qb_h.append(np.ascontiguousarray(
            (q_b[sl] * SCALE * WS * XS).reshape(4, 128).T).astype(np.float32))

    xq_b, xk_b, xv_b, jb_b = [], [], [], []
    for b in range(B):
        pos = pos_b[b]
        n = len(pos)
        qT = query[b].T * XS  # [D, L]
        xq_b.append(np.ascontiguousarray(
            qT.reshape(8, 128, L)).astype(F8))
        # gathered taps: pad x by one column each side, index pos + k
        kT = np.pad(key[b].T * XS, ((0, 0), (1, 1)))
        vT_ = np.pad(value[b].T * XS, ((0, 0), (1, 1)))
        kg = np.zeros((D, KW, CAP), np.float32)
        vg = np.zeros((D, KW, CAP), np.float32)
        for k in range(KW):
            kg[:, k, :n] = kT[:, pos + k]
            vg[:, k, :n] = vT_[:, pos + k]
        xk_b.append(np.ascontiguousarray(
            kg.reshape(8, 128, KW, CAP)).astype(F8))
        vh, vl = _f8_hl(vg.reshape(8, 128, KW, CAP))
        xv_b.append(np.ascontiguousarray(
            np.stack([vh, vl], axis=2)))  # [8, 128, 2, KW, CAP]
        jb = np.full(CAP, MASK_BIAS, np.float32)
        jb[:n] = am_row[pos]
        jb_b.append(np.ascontiguousarray(
            jb.reshape(capc, 128).T).astype(np.float32))

    in_maps = []
    for c in range(NCORES):
        b, hg = c // 2, c % 2
        in_maps.append({
            "xq": xq_b[b], "xk": xk_b[b], "xv": xv_b[b],
            "wq": wq_h[hg], "wk": wk_h[hg], "wv": wv_h[hg], "wo": wo_h[hg],
            "qb": qb_h[hg], "jb": jb_b[b],
        })
    return in_maps, capc, (o_w, np.asarray(v_b, np.float32), o_b)


def _postprocess(parts, extras):
    """parts: list of 8 arrays [8,128,L] bf16 (x OUTS) -> [B, L, D] f32."""
    o_w, v_b, o_b = extras
    # v-bias contribution through the out conv (attention rows sum to 1):
    # interior columns see all 3 taps, edge columns lose one.
    a_full = o_w.sum(axis=2) @ v_b            # [D]
    a_l0 = a_full - o_w[:, :, 0] @ v_b        # l = 0 loses tap k=0
    a_lL = a_full - o_w[:, :, 2] @ v_b        # l = L-1 loses tap k=2
    inv = 1.0 / OUTS
    out = np.empty((B, L, D), np.float32)
    for b in range(B):
        tot = (np.asarray(parts[2 * b], np.float32)
               + np.asarray(parts[2 * b + 1], np.float32)).reshape(D, L) * inv
        tot = tot + o_b[:, None] + a_full[:, None]
        tot[:, 0] += a_l0 - a_full
        tot[:, -1] += a_lL - a_full
        out[b] = tot.T
    return out


def _run(nc, in_maps, trace=False, **kw):
    from concourse import bass_utils
    try:
        res = bass_utils.run_bass_kernel_spmd(
            nc, in_maps, core_ids=list(range(NCORES)), trace=trace, **kw)
    except ModuleNotFoundError:
        # NTFF profiling hook unavailable (axon client without axon.trn);
        # rerun without trace.
        res = bass_utils.run_bass_kernel_spmd(
            nc, in_maps, core_ids=list(range(NCORES)), trace=False, **kw)
    return res


def kernel(**inputs) -> np.ndarray:
    in_maps, capc, extras = _prep_inputs(**inputs)
    nc = _get_nc(capc)
    res = _run(nc, in_maps,
               trace=bool(int(os.environ.get("KERNEL_TRACE", "0"))))
    parts = [res.results[c]["out"] for c in range(NCORES)]
    out = _postprocess(parts, extras)
    if res.exec_time_ns is not None:
        print(f"HW exec time: {res.exec_time_ns} ns")
    return out
